# revision 24
# baseline (speedup 1.0000x reference)
"""Trainium2 Bass kernel for causal multi-head attention block.

Reference computation (fp32):
    qkv = x @ w_qkv;  q,k,v = split(qkv)
    attn = softmax(causal_mask(q k^T / sqrt(64)))
    out  = (attn @ v reassembled) @ w_out

Sharding over 8 NeuronCores: core c handles batch b = c//4 and heads
4*(c%4) .. 4*(c%4)+3 (4 of 16 heads).  Each core computes the rank-256
partial product of the output projection restricted to its heads'
channels; the host sums the 4 partials per batch.

All inputs stream in as fp16 (halves HBM traffic); accumulation stays
fp32 in PSUM.  The kernel fuses the qkv-projection phase with the
attention phase: projection chains for block t+1 are interleaved as PE
filler inside the attention j-loop of query block t, so the ScalarE exp
stream (the phase-2 rate limiter) starts ~10us into the kernel and the
PE never sits behind it.  A warm-up matmul burst at t=0 releases the
HAM clock gate before real work arrives.

Softmax denominators ride as a fused 65th lhsT column per head (ones at
column 64 for even head slots -> PSUM row 64; ones at column 0 for odd
slots -> output rows 63:128, denominator in row 63), so both heads'
unnormalized outputs land partition-aligned with the final oT stack.
Normalization is all on-chip: DVE reciprocal reads the denominator rows
straight from PSUM, a K=1 ones-matmul broadcasts 1/d across partitions
into a PSUM scratch tile, and two DVE multiplies write the normalized
fp16 oT -- no DRAM round trips.
"""

import sys

for _p in ("/opt/trn_rl_repo", "/root/.axon_site/_ro/trn_rl_repo"):
    if _p not in sys.path:
        sys.path.append(_p)

import numpy as np

import concourse.bass as bass
import concourse.mybir as mybir
import concourse.tile as tile
from concourse import bacc, bass_utils

P = 128
B, T, C = 2, 2048, 1024
HPC = 4            # heads per core
DH = 64            # head dim
KT = C // P        # 8 contraction tiles over d_model
NQB = T // 512     # 4 query blocks of 512
NKT = T // P       # 16 key tiles of 128
F32 = mybir.dt.float32
F16 = mybir.dt.float16
EXP = mybir.ActivationFunctionType.Exp
SCALE = 1.0 / 8.0  # 1/sqrt(DH)


def _body(tc, nc, xT, wq, wk, wv, wo, tri, vones, out):
    with (
        tc.tile_pool(name="cst", bufs=1) as cpool,
        tc.tile_pool(name="xt", bufs=1) as xpool,
        tc.tile_pool(name="ptp", bufs=5) as ptp,
        tc.tile_pool(name="osb", bufs=3) as osb,
        tc.tile_pool(name="nrm", bufs=2) as nrm,
        tc.tile_pool(name="psA", bufs=2, space="PSUM") as psA,  # qkv ring
        tc.tile_pool(name="psB", bufs=2, space="PSUM") as psB,  # sp ring
        tc.tile_pool(name="psC", bufs=1, space="PSUM") as psC,  # op
    ):
        # ---- PE warm-up: release the HAM clock gate during the DMA window
        wsrc = cpool.tile([P, 64], F16, name="wsrc")
        nc.vector.memset(wsrc, 0.0)
        NWARM = 64
        wdst = psA.tile([P, 512], F32, name="wdst", tag="qkv")
        for i in range(NWARM):
            nc.tensor.matmul(
                wdst[0:64, 0:64], wsrc, wsrc, start=(i == 0), stop=(i == NWARM - 1)
            )

        # ---- persistent SBUF ----
        wq_sb = cpool.tile([P, KT, 2 * P], F16, name="wq_sb")
        wk_sb = cpool.tile([P, KT, 2 * P], F16, name="wk_sb")
        wv_sb = cpool.tile([P, KT, 2 * P], F16, name="wv_sb")
        wo_sb = cpool.tile([P, 2, C], F16, name="wo_sb")
        tri_sb = cpool.tile([P, P], F16, name="tri_sb")
        qT = [cpool.tile([P, T], F16, name=f"qT{pr}") for pr in range(2)]
        kT = [cpool.tile([P, T], F16, name=f"kT{pr}") for pr in range(2)]
        # v + fused ones column [v(64) | 1]: O rows 0:65, denominator row 64
        vS = cpool.tile([P, NKT, HPC, DH + 1], F16, name="vS")
        oT = [cpool.tile([P, T], F16, name=f"oT{pr}") for pr in range(2)]
        cones = cpool.tile([65, DH], F32, name="cones")
        nc.vector.memset(cones, 1.0)

        # ---- input DMA: x on the sync queue, weights on the vector queue
        # (parallel streams), bulky later-phase constants via gpsimd SWDGE.
        wqv = wq.rearrange("(kt p) n -> p kt n", p=P)
        wkv = wk.rearrange("(kt p) n -> p kt n", p=P)
        wvv = wv.rearrange("(kt p) n -> p kt n", p=P)
        xTv = xT.rearrange("(kt p) t -> p kt t", p=P)
        xts = [
            xpool.tile([P, KT, 512], F16, name=f"xt{i}", bufs=1) for i in range(NQB)
        ]
        nc.scalar.dma_start(wq_sb[:, 0:4], wqv[:, 0:4])
        nc.sync.dma_start(xts[0][:, 0:4, :], xTv[:, 0:4, 0:512])
        nc.scalar.dma_start(wq_sb[:, 4:8], wqv[:, 4:8])
        nc.sync.dma_start(xts[0][:, 4:8, :], xTv[:, 4:8, 0:512])
        nc.scalar.dma_start(wk_sb, wkv)
        nc.scalar.dma_start(wv_sb, wvv)
        for later in range(1, NQB):
            nc.sync.dma_start(xts[later], xTv[:, :, later * 512 : (later + 1) * 512])
        nc.gpsimd.dma_start(tri_sb, tri)
        nc.gpsimd.dma_start(vS[:, :, :, DH : DH + 1], vones)
        nc.gpsimd.dma_start(wo_sb, wo.rearrange("(g p) n -> p g n", p=P))

        # preload the exp ACT table set during the startup DMA window
        warm = cpool.tile([1, 2], F32, name="warm")
        nc.vector.memset(warm, 0.0)
        nc.scalar.activation(warm, warm, EXP, scale=1.0)

        # ---- phase-1 units (projection chains), used as PE filler ----
        def qk_unit(tb5, pr, w_sb, dst):
            def go():
                qp = psA.tile([P, 512], F32, name="qp", tag="qkv")
                for kt in range(KT):
                    nc.tensor.matmul(
                        qp,
                        w_sb[:, kt, pr * P : (pr + 1) * P],
                        xts[tb5][:, kt, :],
                        start=(kt == 0),
                        stop=(kt == KT - 1),
                    )
                nc.vector.tensor_copy(dst[:, tb5 * 512 : (tb5 + 1) * 512], qp)
            return go

        def v_unit(tb5, sub):
            def go():
                tb1 = tb5 * 4 + sub
                vp = psA.tile([P, 512], F32, name="vp", tag="qkv")
                for kt in range(KT):
                    nc.tensor.matmul(
                        vp[:, 0:256],
                        xts[tb5][:, kt, sub * P : (sub + 1) * P],
                        wv_sb[:, kt, :],
                        start=(kt == 0),
                        stop=(kt == KT - 1),
                    )
                nc.vector.tensor_copy(
                    vS[:, tb1, :, 0:DH],
                    vp[:, 0:256].rearrange("p (h d) -> p h d", d=DH),
                )
            return go

        def ph1_units(tb5):
            us = [qk_unit(tb5, pr, wq_sb, qT[pr]) for pr in range(2)]
            us += [qk_unit(tb5, pr, wk_sb, kT[pr]) for pr in range(2)]
            us += [v_unit(tb5, sub) for sub in range(4)]
            return us

        def proj_unit(qb, tb1):
            def go():
                pp = psB.tile([P, 1024], F32, name="pp", tag="sp")
                for pr in range(2):
                    for cb in range(2):
                        nc.tensor.matmul(
                            pp[:, cb * 512 : (cb + 1) * 512],
                            oT[pr][:, tb1 * P : (tb1 + 1) * P],
                            wo_sb[:, pr, cb * 512 : (cb + 1) * 512],
                            start=(pr == 0),
                            stop=(pr == 1),
                        )
                ot = osb.tile([P, 1024], F16, name="ot")
                nc.vector.tensor_copy(ot, pp)
                nc.sync.dma_start(out[tb1 * P : (tb1 + 1) * P, :], ot)
            return go

        def norm_bcmul(pr, qb, oTu, rr):
            # broadcast 1/den across partitions via a K=1 ones-matmul into a
            # PSUM scratch, then normalize (SBUF x PSUM -> fp16 oT).  The
            # second head's rows land at partitions 0:64, so a small
            # SBUF->SBUF DMA shifts them to oT rows 64:128.
            def go():
                bc = psB.tile([P, 1024], F32, name="bc", tag="sp")
                nc.tensor.matmul(
                    bc[0:DH, 0:512], cones[64:65, :], rr[64:65, 0:512],
                    start=True, stop=True,
                )
                nc.tensor.matmul(
                    bc[0:DH, 512:1024], cones[64:65, :], rr[64:65, 512:1024],
                    start=True, stop=True,
                )
                qs = slice(qb * 512, (qb + 1) * 512)
                nc.vector.tensor_mul(
                    oT[pr][0:DH, qs], oTu[0:DH, 0:512], bc[0:DH, 0:512]
                )
                o1 = nrm.tile([DH, 512], F16, name="o1", tag="o1")
                nc.vector.tensor_mul(o1, oTu[0:DH, 512:1024], bc[0:DH, 512:1024])
                nc.sync.dma_start(oT[pr][DH:P, qs], o1)
            return go

        # ---- attention chains with interleaved filler ----
        # fillers: list of (deadline_step, closure).  Consumed in order; a
        # closure runs no later than the start of its deadline step (hard
        # ordering constraints -- the PE executes in emission order), and
        # otherwise at an even cadence across the 2*nk chain steps.
        def attn(qb, fillers):
            nk = 4 * qb + 4
            total = 2 * nk
            nf = len(fillers)
            state = {"consumed": 0, "si": 0}

            def run_filler():
                if state["consumed"] < nf:
                    fillers[state["consumed"]][1]()
                    state["consumed"] += 1

            def fill_to_step():
                want = (state["si"] * nf) // total
                while state["consumed"] < nf and (
                    state["consumed"] < want
                    or (
                        fillers[state["consumed"]][0] is not None
                        and fillers[state["consumed"]][0] <= state["si"]
                    )
                ):
                    run_filler()

            prev_bcmul = None
            results = {}
            for pr in range(2):
                op = psC.tile([P, 1024], F32, name="op", tag="op")
                pts = {}

                def geom(j):
                    r = j - 4 * qb
                    width = 512 - r * P if r >= 0 else 512
                    col0 = r * P if r >= 0 else 0
                    return r, width, col0

                def emit_o(j):
                    _, width, col0 = geom(j)
                    pt = pts.pop(j)
                    for h in range(2):
                        nc.tensor.matmul(
                            op[0 : DH + 1, h * 512 + col0 : h * 512 + col0 + width],
                            vS[:, j, pr * 2 + h, :],
                            pt[:, h * 512 : h * 512 + width],
                            start=(j == 0),
                            stop=(j == nk - 1),
                            skip_group_check=True,
                        )

                for j in range(nk):
                    fill_to_step()
                    if pr == 1 and j == 2 and prev_bcmul is not None:
                        prev_bcmul()
                        prev_bcmul = None
                    r, width, col0 = geom(j)
                    qoff = qb * 512 + col0
                    sp = psB.tile([P, 1024], F32, name="sp", tag="sp")
                    for h in range(2):
                        nc.tensor.matmul(
                            sp[:, h * 512 : h * 512 + width],
                            kT[pr][h * DH : (h + 1) * DH, j * P : (j + 1) * P],
                            qT[pr][h * DH : (h + 1) * DH, qoff : qoff + width],
                            start=True,
                            stop=True,
                        )
                    pt = ptp.tile([P, 1024], F16, name="pt")
                    s3 = sp.rearrange("p (h w) -> p h w", h=2)[:, :, 0:width]
                    p3 = pt.rearrange("p (h w) -> p h w", h=2)[:, :, 0:width]
                    nc.scalar.activation(p3, s3, EXP, scale=SCALE)
                    if r >= 0:
                        for h in range(2):
                            nc.vector.tensor_mul(
                                pt[:, h * 512 : h * 512 + P],
                                pt[:, h * 512 : h * 512 + P],
                                tri_sb,
                            )
                    pts[j] = pt
                    if j > 1:
                        emit_o(j - 2)
                    state["si"] += 1
                if nk > 1:
                    emit_o(nk - 2)
                emit_o(nk - 1)

                # evacuate PSUM (frees the op ring slot) and take 1/den
                oTu = nrm.tile([DH + 1, 1024], F32, name="oTu", tag="oTu")
                nc.vector.tensor_copy(oTu, op[0 : DH + 1, :])
                rr = nrm.tile([65, 1024], F32, name="rr", tag="rr")
                nc.vector.reciprocal(rr[64:65, :], oTu[DH : DH + 1, :])
                if pr == 0:
                    prev_bcmul = norm_bcmul(pr, qb, oTu, rr)
                results[pr] = (oTu, rr)
            while state["consumed"] < nf:
                run_filler()
            return results

        # ---- main fused schedule ----
        for u in ph1_units(0):
            u()
        pending = None  # (pr1 op, rr, qb) awaiting normalization
        for tb5 in range(NQB):
            qb = tb5

            def normf():
                pop, prr, pqb = pending
                return (1, norm_bcmul(1, pqb, pop, prr))

            if qb == 0:
                # q/k/v of block 1; no normalization pending yet
                fillers = [(None, u) for u in ph1_units(1)]
            elif qb == 1:
                u2 = ph1_units(2)
                fillers = (
                    [(None, u2[0]), normf()]
                    + [(None, proj_unit(0, i)) for i in range(4)]
                    + [(None, u) for u in u2[1:]]
                )
            elif qb == 2:
                # q of block 3 only; its k/v chains defer into attn(3)
                fillers = (
                    [(1, qk_unit(3, 0, wq_sb, qT[0])), normf()]
                    + [(None, proj_unit(1, 4 + i)) for i in range(4)]
                    + [(None, qk_unit(3, 1, wq_sb, qT[1]))]
                )
            else:  # qb == 3
                fillers = (
                    [(None, qk_unit(3, 0, wk_sb, kT[0])), normf()]
                    + [(9, qk_unit(3, 1, wk_sb, kT[1]))]
                    + [(10 + sub, v_unit(3, sub)) for sub in range(4)]
                    + [(None, proj_unit(2, 8 + i)) for i in range(3)]
                )
            res = attn(qb, fillers)
            pending = (res[1][0], res[1][1], qb)

        # ---- tail: last normalization + projections ----
        proj_unit(2, 11)()  # reserved filler: covers the last recip latency
        pop, prr, pqb = pending
        norm_bcmul(1, pqb, pop, prr)()
        for tb1 in range(12, 16):
            proj_unit(3, tb1)()


def build_bass():
    nc = bacc.Bacc("TRN2", target_bir_lowering=False, debug=False, num_devices=8)
    xT = nc.dram_tensor("xT", [C, T], F16, kind="ExternalInput").ap()
    wq = nc.dram_tensor("wq", [C, 2 * P], F16, kind="ExternalInput").ap()
    wk = nc.dram_tensor("wk", [C, 2 * P], F16, kind="ExternalInput").ap()
    wv = nc.dram_tensor("wv", [C, 2 * P], F16, kind="ExternalInput").ap()
    wo = nc.dram_tensor("wo", [2 * P, C], F16, kind="ExternalInput").ap()
    tri = nc.dram_tensor("tri", [P, P], F16, kind="ExternalInput").ap()
    vones = nc.dram_tensor("vones", [P, NKT, HPC, 1], F16, kind="ExternalInput").ap()
    out = nc.dram_tensor("out", [T, C], F16, kind="ExternalOutput").ap()
    with tile.TileContext(nc) as tc:
        _body(tc, nc, xT, wq, wk, wv, wo, tri, vones, out)
    nc.compile()
    return nc


def make_in_maps(x, w_qkv, w_out):
    """Host-side sharding: returns the 8 per-core input dicts."""
    x = np.ascontiguousarray(np.asarray(x, dtype=np.float32))
    w_qkv = np.ascontiguousarray(np.asarray(w_qkv, dtype=np.float32))
    w_out = np.ascontiguousarray(np.asarray(w_out, dtype=np.float32))
    kk = np.arange(P)
    tri = (kk[None, :] >= kk[:, None]).astype(np.float16)  # [k, q]: q >= k
    xTb = [np.ascontiguousarray(x[b].T.astype(np.float16)) for b in range(B)]
    in_maps = []
    for c in range(8):
        b = c // 4
        g = c % 4
        h0 = HPC * g * DH  # 256*g
        in_maps.append(
            {
                "xT": xTb[b],
                "wq": np.ascontiguousarray(w_qkv[:, h0 : h0 + 2 * P].astype(np.float16)),
                "wk": np.ascontiguousarray(
                    w_qkv[:, C + h0 : C + h0 + 2 * P].astype(np.float16)
                ),
                "wv": np.ascontiguousarray(
                    w_qkv[:, 2 * C + h0 : 2 * C + h0 + 2 * P].astype(np.float16)
                ),
                "wo": np.ascontiguousarray(w_out[h0 : h0 + 2 * P, :].astype(np.float16)),
                "tri": np.ascontiguousarray(tri),
                "vones": np.ones((P, NKT, HPC, 1), dtype=np.float16),
            }
        )
    return in_maps


_NC_CACHE = None
LAST_RESULTS = None  # BassKernelResults of the most recent run (for profiling)
TRACE = False


def kernel(x, w_qkv, w_out):
    global _NC_CACHE, LAST_RESULTS
    if _NC_CACHE is None:
        _NC_CACHE = build_bass()
    nc = _NC_CACHE
    in_maps = make_in_maps(x, w_qkv, w_out)
    res = bass_utils.run_bass_kernel_spmd(
        nc, in_maps, core_ids=list(range(8)), trace=TRACE
    )
    LAST_RESULTS = res
    partials = [res.results[c]["out"] for c in range(8)]
    out = np.zeros((B, T, C), dtype=np.float32)
    for c in range(8):
        out[c // 4] += partials[c].astype(np.float32)
    return out


if __name__ == "__main__":
    # smoke test with random data
    rng = np.random.default_rng(0)
    x = rng.standard_normal((B, T, C), dtype=np.float32)
    w_qkv = rng.standard_normal((C, 3 * C), dtype=np.float32) / np.sqrt(C)
    w_out = rng.standard_normal((C, C), dtype=np.float32) / np.sqrt(C)
    o = kernel(x, w_qkv, w_out)
    print(o.shape, o.dtype)


# revision 31
# speedup vs baseline: 1.1627x; 1.1627x over previous
"""Trainium2 Bass kernel for causal multi-head attention block.

Reference computation (fp32):
    qkv = x @ w_qkv;  q,k,v = split(qkv)
    attn = softmax(causal_mask(q k^T / sqrt(64)))
    out  = (attn @ v reassembled) @ w_out

Sharding over 8 NeuronCores: core c handles batch b = c//4 and heads
4*(c%4) .. 4*(c%4)+3 (4 of 16 heads).  Each core computes the rank-256
partial product of the output projection restricted to its heads'
channels; the host sums the 4 partials per batch.

All inputs stream in as fp16 (halves HBM traffic); accumulation stays
fp32 in PSUM.  The kernel fuses the qkv-projection phase with the
attention phase: projection chains for block t+1 are interleaved as PE
filler inside the attention j-loop of query block t, so the ScalarE exp
stream (the phase-2 rate limiter) starts ~10us into the kernel and the
PE never sits behind it.  A warm-up matmul burst at t=0 releases the
HAM clock gate before real work arrives.

Softmax denominators ride as a fused 65th lhsT column per head (ones at
column 64 for even head slots -> PSUM row 64; ones at column 0 for odd
slots -> output rows 63:128, denominator in row 63), so both heads'
unnormalized outputs land partition-aligned with the final oT stack.
Normalization is all on-chip: DVE reciprocal reads the denominator rows
straight from PSUM, a K=1 ones-matmul broadcasts 1/d across partitions
into a PSUM scratch tile, and two DVE multiplies write the normalized
fp16 oT -- no DRAM round trips.
"""

import sys

for _p in ("/opt/trn_rl_repo", "/root/.axon_site/_ro/trn_rl_repo"):
    if _p not in sys.path:
        sys.path.append(_p)

import numpy as np

import concourse.bass as bass
import concourse.mybir as mybir
import concourse.tile as tile
from concourse import bacc, bass_utils

P = 128
B, T, C = 2, 2048, 1024
HPC = 4            # heads per core
DH = 64            # head dim
KT = C // P        # 8 contraction tiles over d_model
NQB = T // 512     # 4 query blocks of 512
NKT = T // P       # 16 key tiles of 128
F32 = mybir.dt.float32
F16 = mybir.dt.float16
EXP = mybir.ActivationFunctionType.Exp
SCALE = 1.0 / 8.0  # 1/sqrt(DH)


def _body(tc, nc, xT, wq, wk, wv, wo, tri, vones, out):
    with (
        tc.tile_pool(name="cst", bufs=1) as cpool,
        tc.tile_pool(name="xt", bufs=1) as xpool,
        tc.tile_pool(name="ptp", bufs=5) as ptp,
        tc.tile_pool(name="osb", bufs=3) as osb,
        tc.tile_pool(name="nrm", bufs=2) as nrm,
        tc.tile_pool(name="psA", bufs=2, space="PSUM") as psA,  # qkv ring
        tc.tile_pool(name="psB", bufs=2, space="PSUM") as psB,  # sp ring
        tc.tile_pool(name="psC", bufs=1, space="PSUM") as psC,  # op
    ):
        # ---- PE warm-up: release the HAM clock gate during the DMA window
        wsrc = cpool.tile([P, 64], F16, name="wsrc")
        nc.vector.memset(wsrc, 0.0)
        NWARM = 64
        wdst = psA.tile([P, 512], F32, name="wdst", tag="qkv")
        for i in range(NWARM):
            nc.tensor.matmul(
                wdst[0:64, 0:64], wsrc, wsrc, start=(i == 0), stop=(i == NWARM - 1)
            )

        # ---- persistent SBUF ----
        wq_sb = cpool.tile([P, KT, 2 * P], F16, name="wq_sb")
        wk_sb = cpool.tile([P, KT, 2 * P], F16, name="wk_sb")
        wv_sb = cpool.tile([P, KT, 2 * P], F16, name="wv_sb")
        wo_sb = cpool.tile([P, 2, C], F16, name="wo_sb")
        tri_sb = cpool.tile([P, P], F16, name="tri_sb")
        qT = [cpool.tile([P, T], F16, name=f"qT{pr}") for pr in range(2)]
        kT = [cpool.tile([P, T], F16, name=f"kT{pr}") for pr in range(2)]
        # v + fused ones column [v(64) | 1]: O rows 0:65, denominator row 64
        vS = cpool.tile([P, NKT, HPC, DH + 1], F16, name="vS")
        oT = [cpool.tile([P, T], F16, name=f"oT{pr}") for pr in range(2)]
        cones = cpool.tile([65, DH], F32, name="cones")
        nc.vector.memset(cones, 1.0)

        # ---- input DMA: x on the sync queue, weights on the vector queue
        # (parallel streams), bulky later-phase constants via gpsimd SWDGE.
        wqv = wq.rearrange("(kt p) n -> p kt n", p=P)
        wkv = wk.rearrange("(kt p) n -> p kt n", p=P)
        wvv = wv.rearrange("(kt p) n -> p kt n", p=P)
        xTv = xT.rearrange("(kt p) t -> p kt t", p=P)
        xts = [
            xpool.tile([P, KT, 512], F16, name=f"xt{i}", bufs=1) for i in range(NQB)
        ]
        nc.scalar.dma_start(wq_sb[:, 0:4], wqv[:, 0:4])
        nc.sync.dma_start(xts[0][:, 0:4, :], xTv[:, 0:4, 0:512])
        nc.scalar.dma_start(wq_sb[:, 4:8], wqv[:, 4:8])
        nc.sync.dma_start(xts[0][:, 4:8, :], xTv[:, 4:8, 0:512])
        nc.scalar.dma_start(wk_sb, wkv)
        nc.scalar.dma_start(wv_sb, wvv)
        for later in range(1, NQB):
            nc.sync.dma_start(xts[later], xTv[:, :, later * 512 : (later + 1) * 512])
        nc.gpsimd.dma_start(tri_sb, tri)
        nc.gpsimd.dma_start(vS[:, :, :, DH : DH + 1], vones)
        nc.gpsimd.dma_start(wo_sb, wo.rearrange("(g p) n -> p g n", p=P))

        # preload the exp ACT table set during the startup DMA window
        warm = cpool.tile([1, 2], F32, name="warm")
        nc.vector.memset(warm, 0.0)
        nc.scalar.activation(warm, warm, EXP, scale=1.0)

        # ---- phase-1 units (projection chains), used as PE filler ----
        def qk_unit(tb5, pr, w_sb, dst):
            def go():
                qp = psA.tile([P, 512], F32, name="qp", tag="qkv")
                for kt in range(KT):
                    nc.tensor.matmul(
                        qp,
                        w_sb[:, kt, pr * P : (pr + 1) * P],
                        xts[tb5][:, kt, :],
                        start=(kt == 0),
                        stop=(kt == KT - 1),
                    )
                nc.vector.tensor_copy(dst[:, tb5 * 512 : (tb5 + 1) * 512], qp)
            return go

        def v_unit(tb5, sub):
            def go():
                tb1 = tb5 * 4 + sub
                vp = psA.tile([P, 512], F32, name="vp", tag="qkv")
                for kt in range(KT):
                    nc.tensor.matmul(
                        vp[:, 0:256],
                        xts[tb5][:, kt, sub * P : (sub + 1) * P],
                        wv_sb[:, kt, :],
                        start=(kt == 0),
                        stop=(kt == KT - 1),
                    )
                nc.vector.tensor_copy(
                    vS[:, tb1, :, 0:DH],
                    vp[:, 0:256].rearrange("p (h d) -> p h d", d=DH),
                )
            return go

        def ph1_units(tb5):
            us = [qk_unit(tb5, pr, wq_sb, qT[pr]) for pr in range(2)]
            us += [qk_unit(tb5, pr, wk_sb, kT[pr]) for pr in range(2)]
            us += [v_unit(tb5, sub) for sub in range(4)]
            return us

        def proj_unit(qb, tb1):
            def go():
                pp = psB.tile([P, 1024], F32, name="pp", tag="sp")
                for pr in range(2):
                    for cb in range(2):
                        nc.tensor.matmul(
                            pp[:, cb * 512 : (cb + 1) * 512],
                            oT[pr][:, tb1 * P : (tb1 + 1) * P],
                            wo_sb[:, pr, cb * 512 : (cb + 1) * 512],
                            start=(pr == 0),
                            stop=(pr == 1),
                        )
                ot = osb.tile([P, 1024], F16, name="ot")
                nc.vector.tensor_copy(ot, pp)
                nc.sync.dma_start(out[tb1 * P : (tb1 + 1) * P, :], ot)
            return go

        def norm_bcmul(pr, qb, oTu, rr):
            # broadcast 1/den across partitions via a K=1 ones-matmul into a
            # PSUM scratch, then normalize (SBUF x PSUM -> fp16 oT).  The
            # second head's rows land at partitions 0:64, so a small
            # SBUF->SBUF DMA shifts them to oT rows 64:128.
            def go():
                bc = psB.tile([P, 1024], F32, name="bc", tag="sp")
                nc.tensor.matmul(
                    bc[0:DH, 0:512], cones[64:65, :], rr[64:65, 0:512],
                    start=True, stop=True,
                )
                nc.tensor.matmul(
                    bc[0:DH, 512:1024], cones[64:65, :], rr[64:65, 512:1024],
                    start=True, stop=True,
                )
                qs = slice(qb * 512, (qb + 1) * 512)
                nc.vector.tensor_mul(
                    oT[pr][0:DH, qs], oTu[0:DH, 0:512], bc[0:DH, 0:512]
                )
                o1 = nrm.tile([DH, 512], F16, name="o1", tag="o1")
                nc.vector.tensor_mul(o1, oTu[0:DH, 512:1024], bc[0:DH, 512:1024])
                nc.sync.dma_start(oT[pr][DH:P, qs], o1)
            return go

        # ---- attention chains with interleaved filler ----
        # fillers: list of (deadline_step, closure).  Consumed in order; a
        # closure runs no later than the start of its deadline step (hard
        # ordering constraints -- the PE executes in emission order), and
        # otherwise at an even cadence across the 2*nk chain steps.
        def attn(qb, fillers):
            nk = 4 * qb + 4
            total = 2 * nk
            nf = len(fillers)
            state = {"consumed": 0, "si": 0}

            def run_filler():
                if state["consumed"] < nf:
                    fillers[state["consumed"]][1]()
                    state["consumed"] += 1

            def fill_to_step():
                want = (state["si"] * nf) // total
                while state["consumed"] < nf and (
                    state["consumed"] < want
                    or (
                        fillers[state["consumed"]][0] is not None
                        and fillers[state["consumed"]][0] <= state["si"]
                    )
                ):
                    run_filler()

            results = {}
            for pr in range(2):
                op = psC.tile([P, 1024], F32, name="op", tag="op")
                pts = {}

                def geom(j):
                    r = j - 4 * qb
                    width = 512 - r * P if r >= 0 else 512
                    col0 = r * P if r >= 0 else 0
                    return r, width, col0

                def emit_o(j):
                    _, width, col0 = geom(j)
                    pt = pts.pop(j)
                    for h in range(2):
                        nc.tensor.matmul(
                            op[0 : DH + 1, h * 512 + col0 : h * 512 + col0 + width],
                            vS[:, j, pr * 2 + h, :],
                            pt[:, h * 512 : h * 512 + width],
                            start=(j == 0),
                            stop=(j == nk - 1),
                            skip_group_check=True,
                        )

                for j in range(nk):
                    fill_to_step()
                    r, width, col0 = geom(j)
                    qoff = qb * 512 + col0
                    sp = psB.tile([P, 1024], F32, name="sp", tag="sp")
                    for h in range(2):
                        nc.tensor.matmul(
                            sp[:, h * 512 : h * 512 + width],
                            kT[pr][h * DH : (h + 1) * DH, j * P : (j + 1) * P],
                            qT[pr][h * DH : (h + 1) * DH, qoff : qoff + width],
                            start=True,
                            stop=True,
                        )
                    pt = ptp.tile([P, 1024], F16, name="pt")
                    s3 = sp.rearrange("p (h w) -> p h w", h=2)[:, :, 0:width]
                    p3 = pt.rearrange("p (h w) -> p h w", h=2)[:, :, 0:width]
                    nc.scalar.activation(p3, s3, EXP, scale=SCALE)
                    if r >= 0:
                        # masks run on the (otherwise idle) GpSimd engine so
                        # the DVE queue never gates the exp->O pipeline
                        for h in range(2):
                            nc.gpsimd.tensor_mul(
                                pt[:, h * 512 : h * 512 + P],
                                pt[:, h * 512 : h * 512 + P],
                                tri_sb,
                            )
                    pts[j] = pt
                    if j > 1:
                        emit_o(j - 2)
                    state["si"] += 1
                if nk > 1:
                    emit_o(nk - 2)
                emit_o(nk - 1)

                # evacuate PSUM (frees the op ring slot) and take 1/den
                oTu = nrm.tile([DH + 1, 1024], F32, name="oTu", tag="oTu")
                nc.vector.tensor_copy(oTu, op[0 : DH + 1, :])
                rr = nrm.tile([65, 1024], F32, name="rr", tag="rr")
                nc.vector.reciprocal(rr[64:65, :], oTu[DH : DH + 1, :])
                results[pr] = (oTu, rr)
            while state["consumed"] < nf:
                run_filler()
            return results

        # ---- main fused schedule ----
        for u in ph1_units(0):
            u()
        pending = None  # (qb, {pr: (oTu, rr)}) awaiting normalization
        for tb5 in range(NQB):
            qb = tb5
            norms = []
            if pending is not None:
                pqb, pres = pending
                norms = [
                    (None, norm_bcmul(0, pqb, *pres[0])),
                    (None, norm_bcmul(1, pqb, *pres[1])),
                ]
            if qb == 0:
                # q/k/v of block 1; no normalization pending yet
                fillers = [(None, u) for u in ph1_units(1)]
            elif qb == 1:
                u2 = ph1_units(2)
                fillers = (
                    [(None, u2[0])]
                    + norms
                    + [(None, proj_unit(0, i)) for i in range(4)]
                    + [(None, u) for u in u2[1:]]
                )
            elif qb == 2:
                # q of block 3 only; its k/v chains defer into attn(3)
                fillers = (
                    [(1, qk_unit(3, 0, wq_sb, qT[0]))]
                    + norms
                    + [(None, proj_unit(1, 4 + i)) for i in range(4)]
                    + [(None, qk_unit(3, 1, wq_sb, qT[1]))]
                )
            else:  # qb == 3
                fillers = (
                    [(None, qk_unit(3, 0, wk_sb, kT[0]))]
                    + norms
                    + [(9, qk_unit(3, 1, wk_sb, kT[1]))]
                    + [(10 + sub, v_unit(3, sub)) for sub in range(4)]
                    + [(None, proj_unit(2, 8 + i)) for i in range(3)]
                )
            res = attn(qb, fillers)
            pending = (qb, res)

        # ---- tail: last normalization + projections ----
        proj_unit(2, 11)()  # reserved filler: covers the last recip latency
        pqb, pres = pending
        norm_bcmul(0, pqb, *pres[0])()
        norm_bcmul(1, pqb, *pres[1])()
        for tb1 in range(12, 16):
            proj_unit(3, tb1)()


def build_bass():
    nc = bacc.Bacc("TRN2", target_bir_lowering=False, debug=False, num_devices=8)
    xT = nc.dram_tensor("xT", [C, T], F16, kind="ExternalInput").ap()
    wq = nc.dram_tensor("wq", [C, 2 * P], F16, kind="ExternalInput").ap()
    wk = nc.dram_tensor("wk", [C, 2 * P], F16, kind="ExternalInput").ap()
    wv = nc.dram_tensor("wv", [C, 2 * P], F16, kind="ExternalInput").ap()
    wo = nc.dram_tensor("wo", [2 * P, C], F16, kind="ExternalInput").ap()
    tri = nc.dram_tensor("tri", [P, P], F16, kind="ExternalInput").ap()
    vones = nc.dram_tensor("vones", [P, NKT, HPC, 1], F16, kind="ExternalInput").ap()
    out = nc.dram_tensor("out", [T, C], F16, kind="ExternalOutput").ap()
    with tile.TileContext(nc) as tc:
        _body(tc, nc, xT, wq, wk, wv, wo, tri, vones, out)
    nc.compile()
    return nc


def make_in_maps(x, w_qkv, w_out):
    """Host-side sharding: returns the 8 per-core input dicts."""
    x = np.ascontiguousarray(np.asarray(x, dtype=np.float32))
    w_qkv = np.ascontiguousarray(np.asarray(w_qkv, dtype=np.float32))
    w_out = np.ascontiguousarray(np.asarray(w_out, dtype=np.float32))
    kk = np.arange(P)
    tri = (kk[None, :] >= kk[:, None]).astype(np.float16)  # [k, q]: q >= k
    xTb = [np.ascontiguousarray(x[b].T.astype(np.float16)) for b in range(B)]
    in_maps = []
    for c in range(8):
        b = c // 4
        g = c % 4
        h0 = HPC * g * DH  # 256*g
        in_maps.append(
            {
                "xT": xTb[b],
                "wq": np.ascontiguousarray(w_qkv[:, h0 : h0 + 2 * P].astype(np.float16)),
                "wk": np.ascontiguousarray(
                    w_qkv[:, C + h0 : C + h0 + 2 * P].astype(np.float16)
                ),
                "wv": np.ascontiguousarray(
                    w_qkv[:, 2 * C + h0 : 2 * C + h0 + 2 * P].astype(np.float16)
                ),
                "wo": np.ascontiguousarray(w_out[h0 : h0 + 2 * P, :].astype(np.float16)),
                "tri": np.ascontiguousarray(tri),
                "vones": np.ones((P, NKT, HPC, 1), dtype=np.float16),
            }
        )
    return in_maps


_NC_CACHE = None
LAST_RESULTS = None  # BassKernelResults of the most recent run (for profiling)
TRACE = False


def kernel(x, w_qkv, w_out):
    global _NC_CACHE, LAST_RESULTS
    if _NC_CACHE is None:
        _NC_CACHE = build_bass()
    nc = _NC_CACHE
    in_maps = make_in_maps(x, w_qkv, w_out)
    res = bass_utils.run_bass_kernel_spmd(
        nc, in_maps, core_ids=list(range(8)), trace=TRACE
    )
    LAST_RESULTS = res
    partials = [res.results[c]["out"] for c in range(8)]
    out = np.zeros((B, T, C), dtype=np.float32)
    for c in range(8):
        out[c // 4] += partials[c].astype(np.float32)
    return out


if __name__ == "__main__":
    # smoke test with random data
    rng = np.random.default_rng(0)
    x = rng.standard_normal((B, T, C), dtype=np.float32)
    w_qkv = rng.standard_normal((C, 3 * C), dtype=np.float32) / np.sqrt(C)
    w_out = rng.standard_normal((C, C), dtype=np.float32) / np.sqrt(C)
    o = kernel(x, w_qkv, w_out)
    print(o.shape, o.dtype)


# revision 33
# speedup vs baseline: 1.3423x; 1.1544x over previous
"""Trainium2 Bass kernel for causal multi-head attention block.

Reference computation (fp32):
    qkv = x @ w_qkv;  q,k,v = split(qkv)
    attn = softmax(causal_mask(q k^T / sqrt(64)))
    out  = (attn @ v reassembled) @ w_out

Sharding over 8 NeuronCores: core c handles batch b = c//4 and heads
4*(c%4) .. 4*(c%4)+3 (4 of 16 heads).  Each core computes the rank-256
partial product of the output projection restricted to its heads'
channels; the host sums the 4 partials per batch.

All inputs stream in as fp16 (halves HBM traffic); accumulation stays
fp32 in PSUM.  The kernel fuses the qkv-projection phase with the
attention phase: projection chains for block t+1 are interleaved as PE
filler inside the attention j-loop of query block t, so the ScalarE exp
stream (the phase-2 rate limiter) starts ~10us into the kernel and the
PE never sits behind it.  A warm-up matmul burst at t=0 releases the
HAM clock gate before real work arrives.

Softmax denominators ride as a fused 65th lhsT column per head (ones at
column 64 for even head slots -> PSUM row 64; ones at column 0 for odd
slots -> output rows 63:128, denominator in row 63), so both heads'
unnormalized outputs land partition-aligned with the final oT stack.
Normalization is all on-chip: DVE reciprocal reads the denominator rows
straight from PSUM, a K=1 ones-matmul broadcasts 1/d across partitions
into a PSUM scratch tile, and two DVE multiplies write the normalized
fp16 oT -- no DRAM round trips.
"""

import sys

for _p in ("/opt/trn_rl_repo", "/root/.axon_site/_ro/trn_rl_repo"):
    if _p not in sys.path:
        sys.path.append(_p)

import numpy as np

import concourse.bass as bass
import concourse.mybir as mybir
import concourse.tile as tile
from concourse import bacc, bass_utils

P = 128
B, T, C = 2, 2048, 1024
HPC = 4            # heads per core
DH = 64            # head dim
KT = C // P        # 8 contraction tiles over d_model
NQB = T // 512     # 4 query blocks of 512
NKT = T // P       # 16 key tiles of 128
F32 = mybir.dt.float32
F16 = mybir.dt.float16
EXP = mybir.ActivationFunctionType.Exp
SCALE = 1.0 / 8.0  # 1/sqrt(DH)


def _body(tc, nc, xT, wq, wk, wv, wo, tri, vones, out):
    with (
        tc.tile_pool(name="cst", bufs=1) as cpool,
        tc.tile_pool(name="xt", bufs=1) as xpool,
        tc.tile_pool(name="ptp", bufs=5) as ptp,
        tc.tile_pool(name="osb", bufs=3) as osb,
        tc.tile_pool(name="nrm", bufs=2) as nrm,
        tc.tile_pool(name="psA", bufs=2, space="PSUM") as psA,  # qkv ring
        tc.tile_pool(name="psB", bufs=2, space="PSUM") as psB,  # sp ring
        tc.tile_pool(name="psC", bufs=1, space="PSUM") as psC,  # op
    ):
        # ---- PE warm-up: release the HAM clock gate during the DMA window
        wsrc = cpool.tile([P, 64], F16, name="wsrc")
        nc.vector.memset(wsrc, 0.0)
        NWARM = 64
        wdst = psA.tile([P, 512], F32, name="wdst", tag="qkv")
        for i in range(NWARM):
            nc.tensor.matmul(
                wdst[0:64, 0:64], wsrc, wsrc, start=(i == 0), stop=(i == NWARM - 1)
            )

        # ---- persistent SBUF ----
        wq_sb = cpool.tile([P, KT, 2 * P], F16, name="wq_sb")
        wk_sb = cpool.tile([P, KT, 2 * P], F16, name="wk_sb")
        wv_sb = cpool.tile([P, KT, 2 * P], F16, name="wv_sb")
        wo_sb = cpool.tile([P, 2, C], F16, name="wo_sb")
        tri_sb = cpool.tile([P, P], F16, name="tri_sb")
        qT = [cpool.tile([P, T], F16, name=f"qT{pr}") for pr in range(2)]
        kT = [cpool.tile([P, T], F16, name=f"kT{pr}") for pr in range(2)]
        # v + fused ones column [v(64) | 1]: O rows 0:65, denominator row 64
        vS = cpool.tile([P, NKT, HPC, DH + 1], F16, name="vS")
        oT = [cpool.tile([P, T], F16, name=f"oT{pr}") for pr in range(2)]
        cones = cpool.tile([65, DH], F32, name="cones")
        nc.vector.memset(cones, 1.0)

        # ---- input DMA: x on the sync queue, weights on the vector queue
        # (parallel streams), bulky later-phase constants via gpsimd SWDGE.
        wqv = wq.rearrange("(kt p) n -> p kt n", p=P)
        wkv = wk.rearrange("(kt p) n -> p kt n", p=P)
        wvv = wv.rearrange("(kt p) n -> p kt n", p=P)
        xTv = xT.rearrange("(kt p) t -> p kt t", p=P)
        xts = [
            xpool.tile([P, KT, 512], F16, name=f"xt{i}", bufs=1) for i in range(NQB)
        ]
        nc.scalar.dma_start(wq_sb[:, 0:4], wqv[:, 0:4])
        nc.sync.dma_start(xts[0][:, 0:4, :], xTv[:, 0:4, 0:512])
        nc.scalar.dma_start(wq_sb[:, 4:8], wqv[:, 4:8])
        nc.sync.dma_start(xts[0][:, 4:8, :], xTv[:, 4:8, 0:512])
        nc.scalar.dma_start(wk_sb, wkv)
        nc.scalar.dma_start(wv_sb, wvv)
        for later in range(1, NQB):
            nc.sync.dma_start(xts[later], xTv[:, :, later * 512 : (later + 1) * 512])
        nc.gpsimd.dma_start(tri_sb, tri)
        nc.gpsimd.dma_start(vS[:, :, :, DH : DH + 1], vones)
        nc.gpsimd.dma_start(wo_sb, wo.rearrange("(g p) n -> p g n", p=P))

        # preload the exp ACT table set during the startup DMA window
        warm = cpool.tile([1, 2], F32, name="warm")
        nc.vector.memset(warm, 0.0)
        nc.scalar.activation(warm, warm, EXP, scale=1.0)

        # ---- phase-1 units (projection chains), used as PE filler ----
        def qk_unit(tb5, pr, w_sb, dst):
            def go():
                qp = psA.tile([P, 512], F32, name="qp", tag="qkv")
                for kt in range(KT):
                    nc.tensor.matmul(
                        qp,
                        w_sb[:, kt, pr * P : (pr + 1) * P],
                        xts[tb5][:, kt, :],
                        start=(kt == 0),
                        stop=(kt == KT - 1),
                    )
                nc.vector.tensor_copy(dst[:, tb5 * 512 : (tb5 + 1) * 512], qp)
            return go

        def v_unit(tb5, sub):
            def go():
                tb1 = tb5 * 4 + sub
                vp = psA.tile([P, 512], F32, name="vp", tag="qkv")
                for kt in range(KT):
                    nc.tensor.matmul(
                        vp[:, 0:256],
                        xts[tb5][:, kt, sub * P : (sub + 1) * P],
                        wv_sb[:, kt, :],
                        start=(kt == 0),
                        stop=(kt == KT - 1),
                    )
                nc.vector.tensor_copy(
                    vS[:, tb1, :, 0:DH],
                    vp[:, 0:256].rearrange("p (h d) -> p h d", d=DH),
                )
            return go

        def ph1_units(tb5):
            us = [qk_unit(tb5, pr, wq_sb, qT[pr]) for pr in range(2)]
            us += [qk_unit(tb5, pr, wk_sb, kT[pr]) for pr in range(2)]
            us += [v_unit(tb5, sub) for sub in range(4)]
            return us

        def proj_unit(qb, tb1):
            def go():
                pp = psB.tile([P, 1024], F32, name="pp", tag="sp")
                for pr in range(2):
                    for cb in range(2):
                        nc.tensor.matmul(
                            pp[:, cb * 512 : (cb + 1) * 512],
                            oT[pr][:, tb1 * P : (tb1 + 1) * P],
                            wo_sb[:, pr, cb * 512 : (cb + 1) * 512],
                            start=(pr == 0),
                            stop=(pr == 1),
                        )
                ot = osb.tile([P, 1024], F16, name="ot")
                nc.vector.tensor_copy(ot, pp)
                nc.sync.dma_start(out[tb1 * P : (tb1 + 1) * P, :], ot)
            return go

        def norm_bcmul(pr, qb, oTu, rr):
            # broadcast 1/den across partitions via a K=1 ones-matmul into a
            # PSUM scratch, then normalize (SBUF x PSUM -> fp16 oT).  The
            # second head's rows land at partitions 0:64, so a small
            # SBUF->SBUF DMA shifts them to oT rows 64:128.
            def go():
                bc = psB.tile([P, 1024], F32, name="bc", tag="sp")
                nc.tensor.matmul(
                    bc[0:DH, 0:512], cones[64:65, :], rr[64:65, 0:512],
                    start=True, stop=True,
                )
                nc.tensor.matmul(
                    bc[0:DH, 512:1024], cones[64:65, :], rr[64:65, 512:1024],
                    start=True, stop=True,
                )
                qs = slice(qb * 512, (qb + 1) * 512)
                nc.vector.tensor_mul(
                    oT[pr][0:DH, qs], oTu[0:DH, 0:512], bc[0:DH, 0:512]
                )
                o1 = nrm.tile([DH, 512], F16, name="o1", tag="o1")
                nc.vector.tensor_mul(o1, oTu[0:DH, 512:1024], bc[0:DH, 512:1024])
                nc.sync.dma_start(oT[pr][DH:P, qs], o1)
            return go

        # ---- attention chains with interleaved filler ----
        # fillers: list of (deadline_step, closure).  Consumed in order; a
        # closure runs no later than the start of its deadline step (hard
        # ordering constraints -- the PE executes in emission order), and
        # otherwise at an even cadence across the 2*nk chain steps.
        def attn(qb, fillers):
            nk = 4 * qb + 4
            total = 2 * nk
            nf = len(fillers)
            state = {"consumed": 0, "si": 0}

            def run_filler():
                if state["consumed"] < nf:
                    fillers[state["consumed"]][1]()
                    state["consumed"] += 1

            def fill_to_step():
                want = (state["si"] * nf) // total
                while state["consumed"] < nf and (
                    state["consumed"] < want
                    or (
                        fillers[state["consumed"]][0] is not None
                        and fillers[state["consumed"]][0] <= state["si"]
                    )
                ):
                    run_filler()

            results = {}
            for pr in range(2):
                op = psC.tile([P, 1024], F32, name="op", tag="op")
                pts = {}

                def geom(j):
                    r = j - 4 * qb
                    width = 512 - r * P if r >= 0 else 512
                    col0 = r * P if r >= 0 else 0
                    return r, width, col0

                def emit_o(j):
                    _, width, col0 = geom(j)
                    pt = pts.pop(j)
                    for h in range(2):
                        nc.tensor.matmul(
                            op[0 : DH + 1, h * 512 + col0 : h * 512 + col0 + width],
                            vS[:, j, pr * 2 + h, :],
                            pt[:, h * 512 : h * 512 + width],
                            start=(j == 0),
                            stop=(j == nk - 1),
                            skip_group_check=True,
                        )

                for j in range(nk):
                    fill_to_step()
                    r, width, col0 = geom(j)
                    qoff = qb * 512 + col0
                    sp = psB.tile([P, 1024], F32, name="sp", tag="sp")
                    for h in range(2):
                        nc.tensor.matmul(
                            sp[:, h * 512 : h * 512 + width],
                            kT[pr][h * DH : (h + 1) * DH, j * P : (j + 1) * P],
                            qT[pr][h * DH : (h + 1) * DH, qoff : qoff + width],
                            start=True,
                            stop=True,
                        )
                    pt = ptp.tile([P, 1024], F16, name="pt")
                    s3 = sp.rearrange("p (h w) -> p h w", h=2)[:, :, 0:width]
                    p3 = pt.rearrange("p (h w) -> p h w", h=2)[:, :, 0:width]
                    nc.scalar.activation(p3, s3, EXP, scale=SCALE)
                    if r >= 0:
                        # masks run on the (otherwise idle) GpSimd engine so
                        # the DVE queue never gates the exp->O pipeline
                        for h in range(2):
                            nc.gpsimd.tensor_mul(
                                pt[:, h * 512 : h * 512 + P],
                                pt[:, h * 512 : h * 512 + P],
                                tri_sb,
                            )
                    pts[j] = pt
                    if j > 1:
                        emit_o(j - 2)
                    state["si"] += 1
                if nk > 1:
                    emit_o(nk - 2)
                emit_o(nk - 1)

                # evacuate PSUM (frees the op ring slot) and take 1/den.
                # The reciprocal runs 128 lanes wide: scatter the [1,1024]
                # denominator row to [128,8], invert, gather back -- ~60x
                # cheaper on DVE than a single-partition reciprocal.
                oTu = nrm.tile([DH + 1, 1024], F32, name="oTu", tag="oTu")
                nc.vector.tensor_copy(oTu, op[0 : DH + 1, :])
                rsh = nrm.tile([P, 8], F32, name="rsh", tag="rsh")
                nc.sync.dma_start(rsh, oTu[DH : DH + 1, :])
                rw = nrm.tile([P, 8], F32, name="rw", tag="rw")
                nc.vector.reciprocal(rw, rsh)
                rr = nrm.tile([65, 1024], F32, name="rr", tag="rr")
                nc.sync.dma_start(rr[64:65, :], rw)
                results[pr] = (oTu, rr)
            while state["consumed"] < nf:
                run_filler()
            return results

        # ---- main fused schedule ----
        for u in ph1_units(0):
            u()
        pending = None  # (qb, {pr: (oTu, rr)}) awaiting normalization
        for tb5 in range(NQB):
            qb = tb5
            norms = []
            if pending is not None:
                pqb, pres = pending
                norms = [
                    (None, norm_bcmul(0, pqb, *pres[0])),
                    (None, norm_bcmul(1, pqb, *pres[1])),
                ]
            if qb == 0:
                # q/k/v of block 1; no normalization pending yet
                fillers = [(None, u) for u in ph1_units(1)]
            elif qb == 1:
                u2 = ph1_units(2)
                fillers = (
                    [(None, u2[0])]
                    + norms
                    + [(None, proj_unit(0, i)) for i in range(4)]
                    + [(None, u) for u in u2[1:]]
                )
            elif qb == 2:
                # q of block 3 only; its k/v chains defer into attn(3)
                fillers = (
                    [(1, qk_unit(3, 0, wq_sb, qT[0]))]
                    + norms
                    + [(None, proj_unit(1, 4 + i)) for i in range(4)]
                    + [(None, qk_unit(3, 1, wq_sb, qT[1]))]
                )
            else:  # qb == 3
                fillers = (
                    [(None, qk_unit(3, 0, wk_sb, kT[0]))]
                    + norms
                    + [(9, qk_unit(3, 1, wk_sb, kT[1]))]
                    + [(10 + sub, v_unit(3, sub)) for sub in range(4)]
                    + [(None, proj_unit(2, 8 + i)) for i in range(3)]
                )
            res = attn(qb, fillers)
            pending = (qb, res)

        # ---- tail: last normalization + projections ----
        proj_unit(2, 11)()  # reserved filler: covers the last recip latency
        pqb, pres = pending
        norm_bcmul(0, pqb, *pres[0])()
        norm_bcmul(1, pqb, *pres[1])()
        for tb1 in range(12, 16):
            proj_unit(3, tb1)()


def build_bass():
    nc = bacc.Bacc("TRN2", target_bir_lowering=False, debug=False, num_devices=8)
    xT = nc.dram_tensor("xT", [C, T], F16, kind="ExternalInput").ap()
    wq = nc.dram_tensor("wq", [C, 2 * P], F16, kind="ExternalInput").ap()
    wk = nc.dram_tensor("wk", [C, 2 * P], F16, kind="ExternalInput").ap()
    wv = nc.dram_tensor("wv", [C, 2 * P], F16, kind="ExternalInput").ap()
    wo = nc.dram_tensor("wo", [2 * P, C], F16, kind="ExternalInput").ap()
    tri = nc.dram_tensor("tri", [P, P], F16, kind="ExternalInput").ap()
    vones = nc.dram_tensor("vones", [P, NKT, HPC, 1], F16, kind="ExternalInput").ap()
    out = nc.dram_tensor("out", [T, C], F16, kind="ExternalOutput").ap()
    with tile.TileContext(nc) as tc:
        _body(tc, nc, xT, wq, wk, wv, wo, tri, vones, out)
    nc.compile()
    return nc


def make_in_maps(x, w_qkv, w_out):
    """Host-side sharding: returns the 8 per-core input dicts."""
    x = np.ascontiguousarray(np.asarray(x, dtype=np.float32))
    w_qkv = np.ascontiguousarray(np.asarray(w_qkv, dtype=np.float32))
    w_out = np.ascontiguousarray(np.asarray(w_out, dtype=np.float32))
    kk = np.arange(P)
    tri = (kk[None, :] >= kk[:, None]).astype(np.float16)  # [k, q]: q >= k
    xTb = [np.ascontiguousarray(x[b].T.astype(np.float16)) for b in range(B)]
    in_maps = []
    for c in range(8):
        b = c // 4
        g = c % 4
        h0 = HPC * g * DH  # 256*g
        in_maps.append(
            {
                "xT": xTb[b],
                "wq": np.ascontiguousarray(w_qkv[:, h0 : h0 + 2 * P].astype(np.float16)),
                "wk": np.ascontiguousarray(
                    w_qkv[:, C + h0 : C + h0 + 2 * P].astype(np.float16)
                ),
                "wv": np.ascontiguousarray(
                    w_qkv[:, 2 * C + h0 : 2 * C + h0 + 2 * P].astype(np.float16)
                ),
                "wo": np.ascontiguousarray(w_out[h0 : h0 + 2 * P, :].astype(np.float16)),
                "tri": np.ascontiguousarray(tri),
                "vones": np.ones((P, NKT, HPC, 1), dtype=np.float16),
            }
        )
    return in_maps


_NC_CACHE = None
LAST_RESULTS = None  # BassKernelResults of the most recent run (for profiling)
TRACE = False


def kernel(x, w_qkv, w_out):
    global _NC_CACHE, LAST_RESULTS
    if _NC_CACHE is None:
        _NC_CACHE = build_bass()
    nc = _NC_CACHE
    in_maps = make_in_maps(x, w_qkv, w_out)
    res = bass_utils.run_bass_kernel_spmd(
        nc, in_maps, core_ids=list(range(8)), trace=TRACE
    )
    LAST_RESULTS = res
    partials = [res.results[c]["out"] for c in range(8)]
    out = np.zeros((B, T, C), dtype=np.float32)
    for c in range(8):
        out[c // 4] += partials[c].astype(np.float32)
    return out


if __name__ == "__main__":
    # smoke test with random data
    rng = np.random.default_rng(0)
    x = rng.standard_normal((B, T, C), dtype=np.float32)
    w_qkv = rng.standard_normal((C, 3 * C), dtype=np.float32) / np.sqrt(C)
    w_out = rng.standard_normal((C, C), dtype=np.float32) / np.sqrt(C)
    o = kernel(x, w_qkv, w_out)
    print(o.shape, o.dtype)
